# revision 8
# baseline (speedup 1.0000x reference)
"""Trainium2 Bass kernel for the per-node adaptive output layer (gnn_message_passing).

Computation (per node n):
    w1[n] = sum_c label[n,c] * pool1[c]          (64x32)
    w2[n] = sum_c label[n,c] * pool2[c]          (32x12)
    h     = relu(x[:, n, :]) @ w1[n]             (192x64 @ 64x32)
    out   = relu(h) @ w2[n]                      (192x32 @ 32x12)

Distribution: shard N=2048 nodes across 8 NeuronCores (256 nodes/core), weight
pools + labels replicated (labels sharded with N). No collectives needed.

On-device layout (per core, 256 nodes processed in 16 groups of 16 nodes):
  - x DMA'd with fp32->bf16 cast (SWDGE) into [128, 8*192] tiles:
      partition = 64*(m%2) + d, free col = (m//2)*192 + bt   (m = node-in-group)
  - L1 matmuls packed 8-way into the PE array (64x32 tiling mode),
    L2 matmuls packed 16-way (32x32 mode). bf16 inputs, fp32 PSUM.
  - per-node weights computed on device by small K=8 matmuls from the pools.
"""

import sys
import types

import numpy as np

import concourse.bass as bass
import concourse.mybir as mybir
from concourse import tile
from concourse.bass_utils import run_bass_kernel_spmd


def _ensure_ntff_hook():
    """Register the NTFF profiling hook if the image's antenv lacks it.

    bass_utils' axon trace path imports antenv.axon_hooks unconditionally
    when BASS_TRACE is set; provide it from trn_agent_boot when missing so
    tracing works instead of crashing. Best-effort only.
    """
    try:
        from antenv import axon_hooks  # noqa: F401
        return
    except ImportError:
        pass
    try:
        import antenv
        from trn_agent_boot.trn_boot import _ntff_profile_via_ctypes
        hook = [_ntff_profile_via_ctypes("/opt/axon/libaxon_pjrt.so")]
        mod = types.ModuleType("antenv.axon_hooks")
        mod.get_axon_ntff_profile_hook = lambda: hook[0]
        mod.set_axon_ntff_profile_hook = lambda h: hook.__setitem__(0, h)
        sys.modules["antenv.axon_hooks"] = mod
        antenv.axon_hooks = mod
    except Exception:
        pass


_ensure_ntff_hook()

# Problem shape (hardcoded per harness contract)
B, N, T, D = 16, 2048, 12, 64
C, H, O = 8, 32, 12
NCORES = 8
NSH = N // NCORES            # 256 nodes per core
BT = B * T                   # 192
NGROUPS = 16                 # node groups per core
GN = 16                      # nodes per group
NPAIR = NSH // 2             # 128 node pairs per core (w1sb q index)

FP32 = mybir.dt.float32
BF16 = mybir.dt.bfloat16
RELU = mybir.ActivationFunctionType.Relu

# m = index of node within its group (0..15)
#   p  = m % 2          partition half for L1 (0 -> partitions 0:64, 1 -> 64:128)
#   k8 = m // 2         pair index within group (x free-col block, L1 psum slot)
#   r  = k8 % 4         L2 row group (h1 partition group)
#   u  = p + 2*(k8//4)  L2 output column group / w2 idx sub-index


def _m_of(r, u):
    # inverse map: (r, u) -> m
    p = u % 2
    k8 = r + 4 * (u // 2)
    return 2 * k8 + p


last_exec_time_ns = None
last_results = None
_cached_nc = None


def _build_nc():
    nc = bass.Bass()

    x_ext = nc.declare_dram_parameter(
        "x_dev", [NGROUPS, 128, 8 * BT], FP32, isOutput=False)
    lw1_ext = nc.declare_dram_parameter("label_w1", [C, NSH], FP32, isOutput=False)
    lw2_ext = nc.declare_dram_parameter("label_w2", [C, NSH], FP32, isOutput=False)
    p1_ext = nc.declare_dram_parameter("pool1_t", [C, H * D], FP32, isOutput=False)
    p2_ext = nc.declare_dram_parameter("pool2_t", [C, O * H], FP32, isOutput=False)
    # out layout: [sg, r, u, o, gg, bt]  (sg = g//2, gg = g%2)
    out_ext = nc.declare_dram_parameter(
        "out_dev", [NGROUPS // 2, 4, 4, O, 2, BT], FP32, isOutput=True)

    with tile.TileContext(nc) as tc:
        with tc.tile_pool(name="persist", bufs=1) as persist:
            # per-node weights, bf16, matmul-stationary layout
            w1sb = persist.tile([128, NPAIR * H], BF16)       # [64p+d, q*32+h]
            w2sb = persist.tile([128, (NSH // 4) * O], BF16)  # [32r+k, idx*12+o]
            label1 = persist.tile([C, NSH], BF16)             # cols p*128+q
            label2 = persist.tile([C, NSH], BF16)             # cols r*64+idx
            pool1 = persist.tile([C, H * D], BF16)            # (c, h*64+d)
            pool2 = persist.tile([C, O * H], BF16)            # (c, o*32+k)

            # fp32 -> bf16 cast during DMA (SWDGE)
            nc.gpsimd.dma_start(label1[:], lw1_ext[:])
            nc.gpsimd.dma_start(label2[:], lw2_ext[:])
            nc.gpsimd.dma_start(pool1[:], p1_ext[:])
            nc.gpsimd.dma_start(pool2[:], p2_ext[:])

            # ---------- hypernetwork: per-node weights ----------
            with tc.tile_pool(name="wpsum", bufs=2, space="PSUM") as wpsum:
                # w1sb: for each h, both parities: out[d, q] = sum_c pool1[c,h,d]*label1[c,q]
                for hc in range(H // 4):         # 8 chunks of 4 h values
                    wp = wpsum.tile([128, 512], FP32, tag="wp")
                    for h4 in range(4):
                        h = hc * 4 + h4
                        for p in range(2):
                            nc.tensor.matmul(
                                wp[64 * p:64 * p + 64, h4 * 128:(h4 + 1) * 128],
                                pool1[:, h * D:(h + 1) * D],            # [8, 64]
                                label1[:, p * NPAIR:(p + 1) * NPAIR],   # [8, 128]
                                tile_position=(0, 64 * p),
                            )
                    # permuted copy psum[p, (h4 q)] -> w1sb[p, q*32 + hc*4 + h4]
                    src = wp[:].rearrange("p (h q) -> p q h", h=4)
                    dst = w1sb[:].rearrange("p (q h) -> p q h", h=H)[
                        :, :, hc * 4:(hc + 1) * 4]
                    if hc % 2 == 0:
                        nc.vector.tensor_copy(dst, src)
                    else:
                        nc.scalar.copy(dst, src)

                # w2sb: out[k, idx] = sum_c pool2[c,o,k]*label2[c, r*64+idx]
                for half in range(2):
                    wp2 = wpsum.tile([128, 384], FP32, tag="wp")
                    for o6 in range(6):
                        o = half * 6 + o6
                        for r in range(4):
                            nc.tensor.matmul(
                                wp2[32 * r:32 * r + 32, o6 * 64:(o6 + 1) * 64],
                                pool2[:, o * H:(o + 1) * H],            # [8, 32]
                                label2[:, r * 64:(r + 1) * 64],         # [8, 64]
                                tile_position=(0, 32 * r),
                            )
                    src = wp2[:].rearrange("p (o i) -> p i o", o=6)
                    dst = w2sb[:].rearrange("p (i o) -> p i o", o=O)[
                        :, :, half * 6:(half + 1) * 6]
                    nc.vector.tensor_copy(dst, src)

            # ---------- main loop over 16-node groups ----------
            with (
                tc.tile_pool(name="xin", bufs=4) as xin,
                tc.tile_pool(name="h1p", bufs=4) as h1p,
                tc.tile_pool(name="outp", bufs=8) as outp,
                tc.tile_pool(name="l1ps", bufs=4, space="PSUM") as l1ps,
                tc.tile_pool(name="l2ps", bufs=4, space="PSUM") as l2ps,
            ):
                l2banks = None
                for g in range(NGROUPS):
                    xt = xin.tile([128, 8 * BT], BF16, tag="x")
                    nc.gpsimd.dma_start(xt[:], x_ext[g])      # cast f32->bf16
                    nc.vector.tensor_scalar_max(xt[:], xt[:], 0.0)  # relu(x)

                    # layer 1: 16 matmuls, 8-way PE tiling (64x32)
                    pA = l1ps.tile([128, 384], FP32, tag="l1")  # even (p=0) nodes
                    pB = l1ps.tile([128, 384], FP32, tag="l1")  # odd  (p=1) nodes
                    for m in range(GN):
                        p, k8 = m % 2, m // 2
                        q = g * 8 + k8
                        j, off = k8 % 4, BT * (k8 // 4)
                        dst = pA if p == 0 else pB
                        nc.tensor.matmul(
                            dst[32 * j:32 * j + 32, off:off + BT],
                            w1sb[64 * p:64 * p + 64, q * H:(q + 1) * H],
                            xt[64 * p:64 * p + 64, k8 * BT:(k8 + 1) * BT],
                            tile_position=(64 * p, 32 * j),
                        )

                    # relu + cast to bf16, psum -> sbuf
                    h1A = h1p.tile([128, 384], BF16, tag="h1")
                    h1B = h1p.tile([128, 384], BF16, tag="h1")
                    nc.scalar.activation(h1A[:], pA[:], RELU)
                    nc.scalar.activation(h1B[:], pB[:], RELU)

                    # layer 2: 16 matmuls, 16-way PE tiling (32x32)
                    if g % 2 == 0:
                        l2banks = [
                            l2ps.tile([128, 384], FP32, tag="l2", name=f"l2b{r}")
                            for r in range(4)]
                    for m in range(GN):
                        p, k8 = m % 2, m // 2
                        r, u = k8 % 4, (m % 2) + 2 * (k8 // 4)
                        idx = g * 4 + u
                        src = h1A if p == 0 else h1B
                        nc.tensor.matmul(
                            l2banks[r][32 * u:32 * u + O,
                                       BT * (g % 2):BT * (g % 2) + BT],
                            w2sb[32 * r:32 * r + 32, idx * O:(idx + 1) * O],
                            src[32 * r:32 * r + 32,
                                BT * (k8 // 4):BT * (k8 // 4) + BT],
                            tile_position=(32 * r, 32 * u),
                        )

                    # every 2 groups: evacuate psum and DMA out
                    if g % 2 == 1:
                        sg = g // 2
                        for r in range(4):
                            ot = outp.tile([128, 384], FP32, tag="out")
                            if r % 2 == 0:
                                nc.vector.tensor_copy(ot[:], l2banks[r][:])
                            else:
                                nc.scalar.copy(ot[:], l2banks[r][:])
                            for u in range(4):
                                nc.sync.dma_start(
                                    out_ext[sg, r, u],
                                    ot[32 * u:32 * u + O, :],
                                )

    nc.finalize()
    _legalize_waits(nc)
    return nc


def _legalize_waits(nc, keep_max=1, nop_max=1):
    """Hoist excess per-instruction semaphore waits onto same-engine NOPs.

    This walrus build rejects instructions carrying more than a couple of
    sync-wait commands ("Too many sync wait commands"). Tile attaches all
    required waits directly to consumer instructions; split them onto
    preceding InstNoOps on the same engine (semantically identical: the
    sequencer performs the waits in order before the real instruction).
    """
    ctr = [0]

    def mknop(engine, waits):
        ctr[0] += 1
        return mybir.InstNoOp(
            name=f"I-whoist-{ctr[0]}", engine=engine, bass_nofuse=True,
            sync_info=mybir.SyncInfo(on_wait=list(waits), on_update=[]))

    for f in nc.m.functions:
        for blk in f.blocks:
            out = []
            for inst in blk.instructions:
                si = getattr(inst, 'sync_info', None)
                eng = getattr(inst, 'engine', None)
                if si is not None and eng is not None and len(si.on_wait) > keep_max:
                    waits = list(si.on_wait)
                    keep, hoist = waits[:keep_max], waits[keep_max:]
                    for i in range(0, len(hoist), nop_max):
                        out.append(mknop(eng, hoist[i:i + nop_max]))
                    inst.sync_info = mybir.SyncInfo(
                        on_wait=keep, on_update=list(si.on_update))
                out.append(inst)
            blk.instructions = out


def _get_nc():
    global _cached_nc
    if _cached_nc is None:
        _cached_nc = _build_nc()
    return _cached_nc


def _prep_inputs(x, node_label, weights_pool1, weights_pool2):
    """Shard + pre-transpose full inputs into per-core in_maps."""
    x = np.ascontiguousarray(x, dtype=np.float32)
    node_label = np.ascontiguousarray(node_label, dtype=np.float32)
    p1 = np.ascontiguousarray(
        weights_pool1.transpose(0, 2, 1), dtype=np.float32).reshape(C, H * D)
    p2 = np.ascontiguousarray(
        weights_pool2.transpose(0, 2, 1), dtype=np.float32).reshape(C, O * H)

    # x -> [n, d, bt]
    x_t = np.ascontiguousarray(x.transpose(1, 3, 0, 2)).reshape(N, D, BT)

    # m index table for (r, u)
    m_arr = np.empty((4, 4), dtype=np.int64)
    for r in range(4):
        for u in range(4):
            m_arr[r, u] = _m_of(r, u)

    in_maps = []
    for k in range(NCORES):
        lab = node_label[k * NSH:(k + 1) * NSH]            # [256, 8]
        xs = x_t[k * NSH:(k + 1) * NSH]                    # [256, 64, 192]
        # x_dev[g, 64p+d, k8*192+bt] = x_t[16g + 2*k8 + p, d, bt]
        xdev = xs.reshape(NGROUPS, 8, 2, D, BT).transpose(0, 2, 3, 1, 4)
        xdev = np.ascontiguousarray(xdev).reshape(NGROUPS, 128, 8 * BT)
        # label_w1[c, p*128+q] = lab[2q+p, c]
        lw1 = np.ascontiguousarray(
            lab.reshape(NPAIR, 2, C).transpose(2, 1, 0)).reshape(C, NSH)
        # label_w2[c, r*64 + 4g + u] = lab[16g + m_arr[r,u], c]
        gidx = (16 * np.arange(NGROUPS))[None, :, None] + m_arr[:, None, :]
        lw2 = np.ascontiguousarray(
            lab[gidx.reshape(-1)].reshape(4, NGROUPS, 4, C)
            .transpose(3, 0, 1, 2)).reshape(C, NSH)
        in_maps.append({
            "x_dev": xdev,
            "label_w1": lw1,
            "label_w2": lw2,
            "pool1_t": p1,
            "pool2_t": p2,
        })
    return in_maps


def _unpack_outputs(results):
    """Per-core out_dev [sg, r, u, o, gg, bt] -> full (B, N, T, O)."""
    m_arr = np.empty((4, 4), dtype=np.int64)
    for r in range(4):
        for u in range(4):
            m_arr[r, u] = _m_of(r, u)

    out = np.empty((B, N, T, O), dtype=np.float32)
    for k in range(NCORES):
        od = np.asarray(results[k]["out_dev"])  # [8, 4, 4, 12, 2, 192]
        od = od.transpose(0, 4, 1, 2, 3, 5)     # [sg, gg, r, u, o, bt]
        # node local index l = 16*(2*sg+gg) + m_arr[r, u]
        sg = np.arange(NGROUPS // 2)[:, None, None, None]
        gg = np.arange(2)[None, :, None, None]
        l_arr = 16 * (2 * sg + gg) + m_arr[None, None, :, :]
        out_core = np.empty((NSH, O, BT), dtype=np.float32)
        out_core[l_arr.reshape(-1)] = od.reshape(-1, O, BT)
        # out[b, n, t, o] = out_core[nl, o, b*T+t]
        oc = out_core.reshape(NSH, O, B, T).transpose(2, 0, 3, 1)
        out[:, k * NSH:(k + 1) * NSH] = oc
    return out


def kernel(x, node_label, weights_pool1, weights_pool2):
    global last_exec_time_ns, last_results
    nc = _get_nc()
    in_maps = _prep_inputs(x, node_label, weights_pool1, weights_pool2)
    res = run_bass_kernel_spmd(nc, in_maps, core_ids=list(range(NCORES)))
    last_exec_time_ns = res.exec_time_ns
    last_results = res
    return _unpack_outputs(res.results)


# revision 22
# speedup vs baseline: 1.6777x; 1.6777x over previous
"""Trainium2 Bass kernel for the per-node adaptive output layer (gnn_message_passing).

Computation (per node n):
    w1[n] = sum_c label[n,c] * pool1[c]          (64x32)
    w2[n] = sum_c label[n,c] * pool2[c]          (32x12)
    h     = relu(x[:, n, :]) @ w1[n]             (192x64 @ 64x32)
    out   = relu(h) @ w2[n]                      (192x32 @ 32x12)

Distribution: shard N=2048 nodes across 8 NeuronCores (256 nodes/core), weight
pools + labels replicated (labels sharded with N). No collectives needed.

On-device layout (per core, 256 nodes processed in 16 groups of 16 nodes):
  - x DMA'd with fp32->bf16 cast (SWDGE) into [128, 8*192] tiles:
      partition = 64*(m%2) + d, free col = (m//2)*192 + bt   (m = node-in-group)
  - L1 matmuls packed 8-way into the PE array (64x32 tiling mode),
    L2 matmuls packed 16-way (32x32 mode). bf16 inputs, fp32 PSUM.
  - per-node weights computed on device by small K=8 matmuls from the pools.
"""

import sys
import types

import numpy as np

import concourse.bass as bass
import concourse.mybir as mybir
from concourse import tile
from concourse.bass_utils import run_bass_kernel_spmd


def _ensure_ntff_hook():
    """Register the NTFF profiling hook if the image's antenv lacks it.

    bass_utils' axon trace path imports antenv.axon_hooks unconditionally
    when BASS_TRACE is set; provide it from trn_agent_boot when missing so
    tracing works instead of crashing. Best-effort only.
    """
    try:
        from antenv import axon_hooks  # noqa: F401
        return
    except ImportError:
        pass
    try:
        import antenv
        from trn_agent_boot.trn_boot import _ntff_profile_via_ctypes
        hook = [_ntff_profile_via_ctypes("/opt/axon/libaxon_pjrt.so")]
        mod = types.ModuleType("antenv.axon_hooks")
        mod.get_axon_ntff_profile_hook = lambda: hook[0]
        mod.set_axon_ntff_profile_hook = lambda h: hook.__setitem__(0, h)
        sys.modules["antenv.axon_hooks"] = mod
        antenv.axon_hooks = mod
    except Exception:
        pass


_ensure_ntff_hook()

# Problem shape (hardcoded per harness contract)
B, N, T, D = 16, 2048, 12, 64
C, H, O = 8, 32, 12
NCORES = 8
NSH = N // NCORES            # 256 nodes per core
BT = B * T                   # 192
NGROUPS = 16                 # node groups per core
GN = 16                      # nodes per group
NPAIR = NSH // 2             # 128 node pairs per core (w1sb q index)

FP32 = mybir.dt.float32
BF16 = mybir.dt.bfloat16
RELU = mybir.ActivationFunctionType.Relu

# m = index of node within its group (0..15)
#   p  = m % 2          partition half for L1 (0 -> partitions 0:64, 1 -> 64:128)
#   k8 = m // 2         pair index within group (x free-col block, L1 psum slot)
#   r  = k8 % 4         L2 row group (h1 partition group)
#   u  = p + 2*(k8//4)  L2 output column group / w2 idx sub-index


def _m_of(r, u):
    # inverse map: (r, u) -> m
    p = u % 2
    k8 = r + 4 * (u // 2)
    return 2 * k8 + p


last_exec_time_ns = None
last_results = None
_cached_nc = None


def _build_nc(legalize=True, sim_init=False):
    nc = bass.Bass()

    # x packed as 4 super-blocks of 4 groups: [sb4, 64p+d, g4*1536 + k8*192 + bt]
    x_ext = nc.declare_dram_parameter(
        "x_dev", [NGROUPS // 4, 128, 4 * 8 * BT], FP32, isOutput=False)
    # pools + labels merged into one small param: cols =
    # pool1 (c,h,d) [0:2048] | pool2 (c,o,k) [2048:2432] |
    # label_w1 [2432:2688] | label_w2 [2688:2944]
    wc_ext = nc.declare_dram_parameter("wconst", [C, 2944], FP32, isOutput=False)
    # out layout: [sg, u, o, r*384 + gg*192 + bt]  (sg = g//2, gg = g%2)
    out_ext = nc.declare_dram_parameter(
        "out_dev", [NGROUPS // 2, 4, O, 4 * 2 * BT], FP32, isOutput=True)

    with tile.TileContext(nc) as tc:
        with tc.tile_pool(name="persist", bufs=1) as persist:
            # per-node weights, bf16, matmul-stationary layout
            w1sb = persist.tile([128, NPAIR * H], BF16)       # [64p+d, q*32+h]
            w2sb = persist.tile([128, (NSH // 4) * O], BF16)  # [32r+k, idx*12+o]
            wconst = persist.tile([C, 2944], BF16)

            # fp32 -> bf16 cast during DMA (SWDGE), one transfer for all consts
            nc.gpsimd.dma_start(wconst[:], wc_ext[:])
            pool1 = wconst[:, 0:2048]                # (c, h*64+d)
            pool2 = wconst[:, 2048:2432]             # (c, o*32+k)
            label1 = wconst[:, 2432:2688]            # cols p*128+q
            label2 = wconst[:, 2688:2944]            # cols r*64+idx

            # ---------- hypernetwork: per-node weights ----------
            with tc.tile_pool(name="wpsum", bufs=2, space="PSUM") as wpsum:
                # w1sb: for each h, both parities: out[d, q] = sum_c pool1[c,h,d]*label1[c,q]
                for hc in range(H // 4):         # 8 chunks of 4 h values
                    wp = wpsum.tile([128, 512], FP32, tag="wp")
                    for h4 in range(4):
                        h = hc * 4 + h4
                        for p in range(2):
                            nc.tensor.matmul(
                                wp[64 * p:64 * p + 64, h4 * 128:(h4 + 1) * 128],
                                pool1[:, h * D:(h + 1) * D],            # [8, 64]
                                label1[:, p * NPAIR:(p + 1) * NPAIR],   # [8, 128]
                                tile_position=(0, 64 * p),
                            )
                    # permuted copy psum[p, (h4 q)] -> w1sb[p, q*32 + hc*4 + h4]
                    src = wp[:].rearrange("p (h q) -> p q h", h=4)
                    dst = w1sb[:].rearrange("p (q h) -> p q h", h=H)[
                        :, :, hc * 4:(hc + 1) * 4]
                    if hc % 2 == 0:
                        nc.vector.tensor_copy(dst, src)
                    else:
                        nc.scalar.copy(dst, src)

                # w2sb: out[k, idx] = sum_c pool2[c,o,k]*label2[c, r*64+idx]
                for half in range(2):
                    wp2 = wpsum.tile([128, 384], FP32, tag="wp")
                    for o6 in range(6):
                        o = half * 6 + o6
                        for r in range(4):
                            nc.tensor.matmul(
                                wp2[32 * r:32 * r + 32, o6 * 64:(o6 + 1) * 64],
                                pool2[:, o * H:(o + 1) * H],            # [8, 32]
                                label2[:, r * 64:(r + 1) * 64],         # [8, 64]
                                tile_position=(0, 32 * r),
                            )
                    src = wp2[:].rearrange("p (o i) -> p i o", o=6)
                    dst = w2sb[:].rearrange("p (i o) -> p i o", o=O)[
                        :, :, half * 6:(half + 1) * 6]
                    nc.vector.tensor_copy(dst, src)

            # ---------- main loop over 16-node groups ----------
            with (
                tc.tile_pool(name="xin", bufs=4) as xin,
                tc.tile_pool(name="h1p", bufs=4) as h1p,
                tc.tile_pool(name="outp", bufs=8) as outp,
                tc.tile_pool(name="l1ps", bufs=4, space="PSUM") as l1ps,
                tc.tile_pool(name="l2ps", bufs=4, space="PSUM") as l2ps,
            ):
                l2banks = None
                xt4 = None
                for g in range(NGROUPS):
                    if g % 4 == 0:
                        xt4 = xin.tile([128, 4 * 8 * BT], BF16, tag="x")
                        nc.gpsimd.dma_start(xt4[:], x_ext[g // 4])  # cast f32->bf16
                        nc.vector.tensor_scalar_max(xt4[:], xt4[:], 0.0)  # relu(x)
                    xt = xt4[:, (g % 4) * 8 * BT:(g % 4 + 1) * 8 * BT]

                    # layer 1: 16 matmuls, 8-way PE tiling (64x32)
                    pA = l1ps.tile([128, 384], FP32, tag="l1")  # even (p=0) nodes
                    pB = l1ps.tile([128, 384], FP32, tag="l1")  # odd  (p=1) nodes
                    for m in range(GN):
                        p, k8 = m % 2, m // 2
                        q = g * 8 + k8
                        j, off = k8 % 4, BT * (k8 // 4)
                        dst = pA if p == 0 else pB
                        nc.tensor.matmul(
                            dst[32 * j:32 * j + 32, off:off + BT],
                            w1sb[64 * p:64 * p + 64, q * H:(q + 1) * H],
                            xt[64 * p:64 * p + 64, k8 * BT:(k8 + 1) * BT],
                            tile_position=(64 * p, 32 * j),
                        )

                    # relu + cast to bf16, psum -> sbuf
                    h1A = h1p.tile([128, 384], BF16, tag="h1")
                    h1B = h1p.tile([128, 384], BF16, tag="h1")
                    nc.scalar.activation(h1A[:], pA[:], RELU)
                    nc.scalar.activation(h1B[:], pB[:], RELU)

                    # layer 2: 16 matmuls, 16-way PE tiling (32x32)
                    if g % 2 == 0:
                        l2banks = [
                            l2ps.tile([128, 384], FP32, tag="l2", name=f"l2b{r}")
                            for r in range(4)]
                        if sim_init:
                            # CoreSim-only: matmuls leave 20 of each 32
                            # partitions unwritten; the full-tile evacuation
                            # copy reads them (harmless on HW, flagged in sim)
                            for bank in l2banks:
                                nc.vector.memset(bank[:], 0.0)
                    for m in range(GN):
                        p, k8 = m % 2, m // 2
                        r, u = k8 % 4, (m % 2) + 2 * (k8 // 4)
                        idx = g * 4 + u
                        src = h1A if p == 0 else h1B
                        nc.tensor.matmul(
                            l2banks[r][32 * u:32 * u + O,
                                       BT * (g % 2):BT * (g % 2) + BT],
                            w2sb[32 * r:32 * r + 32, idx * O:(idx + 1) * O],
                            src[32 * r:32 * r + 32,
                                BT * (k8 // 4):BT * (k8 // 4) + BT],
                            tile_position=(32 * r, 32 * u),
                        )

                    # every 2 groups: evacuate psum and DMA out
                    if g % 2 == 1:
                        sg = g // 2
                        otq = outp.tile([128, 4 * 384], FP32, tag="out")
                        for r in range(4):
                            if r % 2 == 0:
                                nc.vector.tensor_copy(
                                    otq[:, r * 384:(r + 1) * 384], l2banks[r][:])
                            else:
                                nc.scalar.copy(
                                    otq[:, r * 384:(r + 1) * 384], l2banks[r][:])
                        for u in range(4):
                            eng = nc.sync if u % 2 == 0 else nc.scalar
                            eng.dma_start(out_ext[sg, u],
                                          otq[32 * u:32 * u + O, :])

    nc.finalize()
    if legalize:
        _legalize_waits(nc)
    return nc


def _legalize_waits(nc, keep_max=1, nop_max=1):
    """Hoist excess per-instruction semaphore waits onto same-engine NOPs.

    This walrus build rejects instructions carrying more than a couple of
    sync-wait commands ("Too many sync wait commands"). Tile attaches all
    required waits directly to consumer instructions; split them onto
    preceding InstNoOps on the same engine (semantically identical: the
    sequencer performs the waits in order before the real instruction).
    """
    ctr = [0]

    def mknop(engine, waits):
        ctr[0] += 1
        return mybir.InstNoOp(
            name=f"I-whoist-{ctr[0]}", engine=engine, bass_nofuse=True,
            sync_info=mybir.SyncInfo(on_wait=list(waits), on_update=[]))

    for f in nc.m.functions:
        for blk in f.blocks:
            out = []
            for inst in blk.instructions:
                si = getattr(inst, 'sync_info', None)
                eng = getattr(inst, 'engine', None)
                if si is not None and eng is not None and len(si.on_wait) > keep_max:
                    waits = list(si.on_wait)
                    keep, hoist = waits[:keep_max], waits[keep_max:]
                    for i in range(0, len(hoist), nop_max):
                        out.append(mknop(eng, hoist[i:i + nop_max]))
                    inst.sync_info = mybir.SyncInfo(
                        on_wait=keep, on_update=list(si.on_update))
                out.append(inst)
            blk.instructions = out


def _get_nc():
    global _cached_nc
    if _cached_nc is None:
        _cached_nc = _build_nc()
    return _cached_nc


def _prep_inputs(x, node_label, weights_pool1, weights_pool2):
    """Shard + pre-transpose full inputs into per-core in_maps."""
    x = np.ascontiguousarray(x, dtype=np.float32)
    node_label = np.ascontiguousarray(node_label, dtype=np.float32)
    p1 = np.ascontiguousarray(
        weights_pool1.transpose(0, 2, 1), dtype=np.float32).reshape(C, H * D)
    p2 = np.ascontiguousarray(
        weights_pool2.transpose(0, 2, 1), dtype=np.float32).reshape(C, O * H)

    # x -> [n, d, bt]
    x_t = np.ascontiguousarray(x.transpose(1, 3, 0, 2)).reshape(N, D, BT)

    # m index table for (r, u)
    m_arr = np.empty((4, 4), dtype=np.int64)
    for r in range(4):
        for u in range(4):
            m_arr[r, u] = _m_of(r, u)

    in_maps = []
    for k in range(NCORES):
        lab = node_label[k * NSH:(k + 1) * NSH]            # [256, 8]
        xs = x_t[k * NSH:(k + 1) * NSH]                    # [256, 64, 192]
        # x_dev[g, 64p+d, k8*192+bt] = x_t[16g + 2*k8 + p, d, bt]
        xdev = xs.reshape(NGROUPS, 8, 2, D, BT).transpose(0, 2, 3, 1, 4)
        xdev = xdev.reshape(NGROUPS, 128, 8 * BT)
        # pack 4 groups per DMA block: [sb4, part, g4*1536 + c]
        xdev = np.ascontiguousarray(
            xdev.reshape(4, 4, 128, 8 * BT).transpose(0, 2, 1, 3)
        ).reshape(4, 128, 4 * 8 * BT)
        # label_w1[c, p*128+q] = lab[2q+p, c]
        lw1 = lab.reshape(NPAIR, 2, C).transpose(2, 1, 0).reshape(C, NSH)
        # label_w2[c, r*64 + 4g + u] = lab[16g + m_arr[r,u], c]
        gidx = (16 * np.arange(NGROUPS))[None, :, None] + m_arr[:, None, :]
        lw2 = lab[gidx.reshape(-1)].reshape(4, NGROUPS, 4, C) \
            .transpose(3, 0, 1, 2).reshape(C, NSH)
        wconst = np.ascontiguousarray(
            np.concatenate([p1, p2, lw1, lw2], axis=1))    # [8, 2944]
        in_maps.append({"x_dev": xdev, "wconst": wconst})
    return in_maps


def _unpack_outputs(results):
    """Per-core out_dev [sg, r, u, o, gg, bt] -> full (B, N, T, O)."""
    m_arr = np.empty((4, 4), dtype=np.int64)
    for r in range(4):
        for u in range(4):
            m_arr[r, u] = _m_of(r, u)

    out = np.empty((B, N, T, O), dtype=np.float32)
    for k in range(NCORES):
        od = np.asarray(results[k]["out_dev"]).reshape(
            NGROUPS // 2, 4, O, 4, 2, BT)       # [sg, u, o, r, gg, bt]
        od = od.transpose(0, 4, 3, 1, 2, 5)     # [sg, gg, r, u, o, bt]
        # node local index l = 16*(2*sg+gg) + m_arr[r, u]
        sg = np.arange(NGROUPS // 2)[:, None, None, None]
        gg = np.arange(2)[None, :, None, None]
        l_arr = 16 * (2 * sg + gg) + m_arr[None, None, :, :]
        out_core = np.empty((NSH, O, BT), dtype=np.float32)
        out_core[l_arr.reshape(-1)] = od.reshape(-1, O, BT)
        # out[b, n, t, o] = out_core[nl, o, b*T+t]
        oc = out_core.reshape(NSH, O, B, T).transpose(2, 0, 3, 1)
        out[:, k * NSH:(k + 1) * NSH] = oc
    return out


def kernel(x, node_label, weights_pool1, weights_pool2):
    global last_exec_time_ns, last_results
    nc = _get_nc()
    in_maps = _prep_inputs(x, node_label, weights_pool1, weights_pool2)
    res = run_bass_kernel_spmd(nc, in_maps, core_ids=list(range(NCORES)))
    last_exec_time_ns = res.exec_time_ns
    last_results = res
    return _unpack_outputs(res.results)
